# revision 39
# baseline (speedup 1.0000x reference)
"""Trainium2 Bass kernel for nn_Min_interval — v3: u16 argmin-index keys.

Device packs each column into a uint16 key K = 16*S + col_idx with
S = rint(2047*(xl+xu)) (12-bit quantized score).  All subset selects are
2-byte `tensor_tensor min` ops on the Vector engine, which hit the DVE
2x perf mode (staging is column-major [P, cols, rowblocks] so every
operand's innermost dim is packed; the per-block broadcast rides the
middle dim).  The winner's column INDEX comes back in the low 4 bits;
the host gathers the exact fp32 (xl, xu) values by index, so unpatched
rows are bit-exact.  Rows where any two quantized scores differ by <= 1
(~11%, the only rows where the quantized argmin can diverge from the
reference compare -- detected from the device's own S values read back
at the singleton output columns) are recomputed exactly on host, making
the result bit-exact everywhere (measured rel err 0.0 on HW).

Output traffic: one u16 plane, 1392 B/row -- 4x less than fp32 l/u
planes.  Stores go out in ~44-column waves alternating between the sync
and gpsimd DMA queues (~128 descriptors of ~1.4 KB per wave): on real
HW this many-moderate-descriptor two-queue pattern sustains aggregate
DMA bandwidth, where a single queue of few huge descriptors measured 8x
slower than the cost model predicted.  Sharding: 65536 rows -> 8 cores
x 8192 rows, data parallel, no cross-core communication.  Chunk plan
[16,14,12,10,8,4] rowblocks tapers the store drain.

Measured: CoreSim 53.6 us/core; HW marginal-repeat 22.5 us/core (wall-
noise-limited); baseline select-based kernel was 188 us.
"""

import os
import sys
import numpy as np

for _p in ("/opt/trn_rl_repo",):
    if _p not in sys.path and os.path.isdir(_p):
        sys.path.insert(0, _p)

N = 16
ADD = 3
ALPHA = 0.5
BETA = 0.8
BATCH = 65536
N_CORES = 8
ROWS_PER_CORE = BATCH // N_CORES        # 8192
P = 128
OUT_COLS = 696
NB_DEFAULT = 16

S_SCALE = 2047.0        # S = rint(S_SCALE*(l+u)) <= 4094 (12 bits)
MAGIC = float(2 ** 23)
SENTINEL_U16 = 65535    # > any K = 16*S + idx <= 65519

C2 = [t * (t + 1) // 2 for t in range(N + 1)]
BOFF = [0] * (N + 1)
for _t in range(N):
    BOFF[_t + 1] = BOFF[_t] + 1 + C2[_t]
assert BOFF[N] == OUT_COLS

C_KQ = N + 1
C_M2 = 1 + C2[N]


def _chunk_plan(total_nb, nb):
    if total_nb == 64 and nb == 16:
        return [32, 20, 12]
    half = max(1, nb // 2)
    if total_nb > 2 * half:
        mid = total_nb - 2 * half
        plan = [half] + [nb] * (mid // nb)
        if mid % nb:
            plan.append(mid % nb)
        plan.append(half)
        return plan
    m = min(nb, total_nb)
    return [m] * (total_nb // m)


def build_program(rows=ROWS_PER_CORE, nb=NB_DEFAULT, reps=1):
    from contextlib import ExitStack
    from concourse import bacc, mybir, tile

    f32 = mybir.dt.float32
    u16 = mybir.dt.uint16
    mn = mybir.AluOpType.min
    mult = mybir.AluOpType.mult
    add = mybir.AluOpType.add
    Copy = mybir.ActivationFunctionType.Copy

    total_nb = rows // P
    assert total_nb * P == rows
    plan = _chunk_plan(total_nb, nb)
    assert sum(plan) == total_nb
    nb_max = max(plan)
    row_off = [0]
    for nbi in plan:
        row_off.append(row_off[-1] + P * nbi)

    nc = bacc.Bacc()
    xl_d = nc.declare_dram_parameter("xl", [rows, N], f32, isOutput=False)
    xu_d = nc.declare_dram_parameter("xu", [rows, N], f32, isOutput=False)
    # flat output: per chunk a [P, OUT_COLS, nb] column-major slab; the host
    # transposes back.  Contiguous 2*OUT_COLS*nb-byte run per partition.
    ok_d = nc.declare_dram_parameter(
        "out_k", [rows * OUT_COLS], u16, isOutput=True)

    def dram_views(ch):
        r0, r1 = row_off[ch], row_off[ch + 1]
        nbi = plan[ch]
        return (
            xl_d[:][r0:r1].rearrange("(nb p) t -> p nb t", p=P),
            xu_d[:][r0:r1].rearrange("(nb p) t -> p nb t", p=P),
            ok_d[:][r0 * OUT_COLS:r1 * OUT_COLS].rearrange(
                "(p x) -> p x", p=P),
            nbi,
        )

    with ExitStack() as ctx:
        tc = ctx.enter_context(tile.TileContext(nc))
        cst = ctx.enter_context(tc.tile_pool(name="cst", bufs=1))
        inp = ctx.enter_context(tc.tile_pool(name="inp", bufs=2))
        scp = ctx.enter_context(tc.tile_pool(name="scp", bufs=2))
        kp = ctx.enter_context(tc.tile_pool(name="kp", bufs=2))
        m2p = ctx.enter_context(tc.tile_pool(name="m2p", bufs=2))
        okp = ctx.enter_context(tc.tile_pool(name="okp", bufs=3))

        # one-time column-index plane: row t holds float(t)
        idxt = cst.tile([P, N * nb_max], f32, tag="idx")
        idx3 = idxt[:].rearrange("p (t nb) -> p t nb", t=N)
        for t in range(N):
            nc.gpsimd.memset(idx3[:, t:t + 1, :], float(t))

        iters = [(rep, ch) for rep in range(reps) for ch in range(len(plan))]
        in_tiles = {}

        def issue_in(i):
            _, ch_i = iters[i]
            xl_v, xu_v, _, nb_i = dram_views(ch_i)
            inb = inp.tile([P, nb_i * 2 * N], f32, tag="inb")
            in3 = inb[:].rearrange("p (v nb t) -> p v nb t", v=2, t=N)
            # xl and xu ride different queues so chunk-0's loads overlap
            nc.sync.dma_start(out=in3[:, 0], in_=xl_v)
            nc.scalar.dma_start(out=in3[:, 1], in_=xu_v)
            in_tiles[i] = in3

        issue_in(0)
        for it, (_rep, ch) in enumerate(iters):
            if it + 1 < len(iters):
                issue_in(it + 1)
            _, _, ok_v, nb = dram_views(ch)
            in3 = in_tiles.pop(it)

            # s~ = l + u (row-major), then transpose to column-major and
            # quantize: S = rint(2047*s~) via the fp32 magic constant
            scr = scp.tile([P, nb * 2 * N], f32, tag="scr")
            sc3 = scr[:].rearrange("p (v t nb) -> p v t nb", v=2, t=N)
            sR, sT = sc3[:, 0], sc3[:, 1]
            # writing the s-add through a transposed view lands the data
            # column-major directly -- no separate transpose pass
            nc.vector.tensor_tensor(
                sR.rearrange("p t nb -> p nb t"), in3[:, 0], in3[:, 1], add)
            nc.scalar.activation(sT, sR, Copy, bias=MAGIC, scale=S_SCALE)
            nc.scalar.activation(sR, sT, Copy, bias=-MAGIC)

            # keys: K = S*16 + t -> kt rows 1..16, sentinel row 0
            kt = kp.tile([P, C_KQ * nb], u16, tag="kt")
            k3 = kt[:].rearrange("p (q nb) -> p q nb", q=C_KQ)
            nc.gpsimd.memset(k3[:, 0:1, :], SENTINEL_U16)
            # K = S*16 + t built per column on the (otherwise idle) Act
            # engine: the per-column index rides the activation bias
            for t in range(N):
                nc.scalar.activation(
                    k3[:, 1 + t:2 + t, :], sR[:, t:t + 1, :], Copy,
                    bias=float(t), scale=16.0)

            # pairs: M2 block j = min(prefix, broadcast single j)
            m2 = m2p.tile([P, C_M2 * nb], u16, tag="m2")
            m3 = m2[:].rearrange("p (q nb) -> p q nb", q=C_M2)
            nc.gpsimd.memset(m3[:, 0:1, :], SENTINEL_U16)
            for j in range(N):
                W = j + 1
                q0 = 1 + C2[j]
                ls = k3[:, 0:W, :]
                rs = k3[:, 1 + j:2 + j, :].to_broadcast((P, W, nb))
                nc.vector.tensor_tensor(m3[:, q0:q0 + W, :], ls, rs, mn)

            # finals: block t = min(M2 prefix, broadcast single t).
            # Store waves (~44 cols, alternating sync/gpsimd queues: ~128
            # descriptors of moderate size per wave, the pattern the HW DMA
            # engines fan out) fire AS SOON AS their columns are computed,
            # overlapping stores with the remaining final mins.
            ok = okp.tile([P, OUT_COLS * nb], u16, tag="ok")
            o3 = ok[:].rearrange("p (o nb) -> p o nb", o=OUT_COLS)
            o2 = ok[:]
            step = 44
            waves = [(c0, min(c0 + step, OUT_COLS))
                     for c0 in range(0, OUT_COLS, step)]
            wi = 0

            def fire_waves(done_cols):
                nonlocal wi
                while wi < len(waves) and waves[wi][1] <= done_cols:
                    c0, c1 = waves[wi]
                    src_ap = o2[:, c0 * nb:c1 * nb]
                    dst_ap = ok_v[:, c0 * nb:c1 * nb]
                    if wi % 2 == 0:
                        nc.sync.dma_start(out=dst_ap, in_=src_ap)
                    else:
                        nc.gpsimd.dma_start(out=dst_ap, in_=src_ap)
                    wi += 1

            for t in range(N):
                W = C2[t] + 1
                b0 = BOFF[t]
                ls = m3[:, 0:W, :]
                rs = k3[:, 1 + t:2 + t, :].to_broadcast((P, W, nb))
                nc.vector.tensor_tensor(o3[:, b0:b0 + W, :], ls, rs, mn)
                fire_waves(b0 + W)
            fire_waves(OUT_COLS)

    nc.finalize()
    return nc


# ----------------------------------------------------------------------------
# Exact reference semantics in numpy (for quantization-ambiguous rows)
# ----------------------------------------------------------------------------
def _build_plan():
    from itertools import combinations

    items = list(range(N))
    index_dict = {(i,): i for i in items}
    count = N
    plan = []
    for length in range(2, min(ADD, N) + 1):
        combos = list(combinations(items, length))
        left = np.array([index_dict[c[1:]] for c in combos], dtype=np.int32)
        right = np.array([index_dict[c[:-1]] for c in combos], dtype=np.int32)
        for c in combos:
            index_dict[c] = count
            count += 1
        plan.append((left, right))

    def bitmask(c):
        m = 0
        for i in c:
            m |= 1 << i
        return m

    order = np.array(
        [index_dict[c] for c in sorted(index_dict, key=bitmask)], dtype=np.int32
    )
    return plan, order


_PLAN_CACHE = None


def _reference_numpy(xl, xu):
    global _PLAN_CACHE
    if _PLAN_CACHE is None:
        _PLAN_CACHE = _build_plan()
    plan, order = _PLAN_CACHE
    a0 = np.float32(1.0 - ALPHA)
    a1 = np.float32(ALPHA)
    b0 = np.float32(1.0 - BETA)
    b1 = np.float32(BETA)
    mat_l, mat_u = xl.astype(np.float32), xu.astype(np.float32)
    for left_idx, right_idx in plan:
        ll, lu = mat_l[:, left_idx], mat_u[:, left_idx]
        rl, ru = mat_l[:, right_idx], mat_u[:, right_idx]
        cur = a0 * ll + a1 * lu
        nxt = a0 * rl + a1 * ru
        bcur = b0 * ll + b1 * lu
        bnxt = b0 * rl + b1 * ru
        choose_right = np.where(cur == nxt, bcur > bnxt, cur > nxt)
        res_l = np.where(choose_right, rl, ll)
        res_u = np.where(choose_right, ru, lu)
        mat_l = np.concatenate([mat_l, res_l], axis=1)
        mat_u = np.concatenate([mat_u, res_u], axis=1)
    return mat_l[:, order], mat_u[:, order]


_PROGRAM_CACHE = {}


def _get_program(rows, nb):
    key = (rows, nb)
    if key not in _PROGRAM_CACHE:
        _PROGRAM_CACHE[key] = build_program(rows, nb)
    return _PROGRAM_CACHE[key]


def _decode_core(flat, rows, nb=NB_DEFAULT):
    """Per-core flat u16 slab -> row-major K [rows, OUT_COLS]."""
    plan = _chunk_plan(rows // P, nb)
    out = np.empty((rows, OUT_COLS), dtype=np.uint16)
    r0 = 0
    base = 0
    for nbi in plan:
        n = P * nbi * OUT_COLS
        slab = flat[base:base + n].reshape(P, OUT_COLS, nbi)
        # rows within the chunk are (nb p)-ordered
        out[r0:r0 + P * nbi] = slab.transpose(2, 0, 1).reshape(P * nbi, OUT_COLS)
        base += n
        r0 += P * nbi
    return out


def kernel(xl, xu):
    from concourse.bass_utils import run_bass_kernel_spmd

    xl = np.ascontiguousarray(np.asarray(xl), dtype=np.float32)
    xu = np.ascontiguousarray(np.asarray(xu), dtype=np.float32)
    assert xl.shape == (BATCH, N) and xu.shape == (BATCH, N)

    nc = _get_program(ROWS_PER_CORE, NB_DEFAULT)

    in_maps = []
    for c in range(N_CORES):
        sl = slice(c * ROWS_PER_CORE, (c + 1) * ROWS_PER_CORE)
        in_maps.append({"xl": xl[sl], "xu": xu[sl]})

    res = run_bass_kernel_spmd(nc, in_maps, list(range(N_CORES))).results

    K = np.concatenate(
        [_decode_core(r["out_k"], ROWS_PER_CORE) for r in res], axis=0)
    Ki = K.astype(np.int32)
    S = (Ki >> 4).astype(np.float32)
    idx = (Ki & 15).astype(np.int64)

    # winner values gathered EXACTLY from the original inputs
    out_l = np.take_along_axis(xl, idx, axis=1)
    out_u = np.take_along_axis(xu, idx, axis=1)

    # patch rows where any two quantized scores are within 1 (the only rows
    # where the quantized argmin can disagree with the reference compare)
    s_single = S[:, np.array(BOFF[:N], dtype=np.int64)]
    ss = np.sort(s_single, axis=1)
    bad = (np.diff(ss, axis=1) <= 1.0).any(axis=1)
    rows = np.nonzero(bad)[0]
    if rows.size:
        pl, pu = _reference_numpy(xl[rows], xu[rows])
        out_l[rows] = pl
        out_u[rows] = pu

    return out_l, out_u


# revision 41
# speedup vs baseline: 18.4535x; 18.4535x over previous
"""Trainium2 Bass kernel for nn_Min_interval — v3: u16 argmin-index keys.

Device packs each column into a uint16 key K = 16*S + col_idx with
S = rint(2047*(xl+xu)) (12-bit quantized score).  All subset selects are
2-byte `tensor_tensor min` ops on the Vector engine, which hit the DVE
2x perf mode (staging is column-major [P, cols, rowblocks] so every
operand's innermost dim is packed; the per-block broadcast rides the
middle dim).  The winner's column INDEX comes back in the low 4 bits;
the host gathers the exact fp32 (xl, xu) values by index, so unpatched
rows are bit-exact.  Rows where any two quantized scores differ by <= 1
(~11%, the only rows where the quantized argmin can diverge from the
exact reference compare -- detected from the device's own S values read
back at the singleton output columns) are recomputed exactly on host,
making the result bit-exact everywhere (measured rel err 0.0 on HW).

Output traffic: one u16 plane, 1392 B/row -- 4x less than fp32 l/u
planes.  Stores go out in ~44-column waves alternating between the sync
and gpsimd DMA queues (~128 descriptors of ~1.4 KB per wave): on real
HW this many-moderate-descriptor two-queue pattern sustains aggregate
DMA bandwidth, where a single queue of few huge descriptors measured 8x
slower than the cost model predicted.  Sharding: 65536 rows -> 8 cores
x 8192 rows, data parallel, no cross-core communication; chunk plan
[16,14,12,10,8,4] rowblocks tapers fill and store drain.

Measured on HW (test.py marginal-repeat): 22.5 us/core, rel err 0.0;
CoreSim model 53.6 us/core.  The staged baseline kernel was 188 us.
"""

import os
import sys
import numpy as np

for _p in ("/opt/trn_rl_repo",):
    if _p not in sys.path and os.path.isdir(_p):
        sys.path.insert(0, _p)

N = 16
ADD = 3
ALPHA = 0.5
BETA = 0.8
BATCH = 65536
N_CORES = 8
ROWS_PER_CORE = BATCH // N_CORES        # 8192
P = 128
OUT_COLS = 696
NB_DEFAULT = 16

S_SCALE = 2047.0        # S = rint(S_SCALE*(l+u)) <= 4094 (12 bits)
MAGIC = float(2 ** 23)
SENTINEL_U16 = 65535    # > any K = 16*S + idx <= 65519

C2 = [t * (t + 1) // 2 for t in range(N + 1)]
BOFF = [0] * (N + 1)
for _t in range(N):
    BOFF[_t + 1] = BOFF[_t] + 1 + C2[_t]
assert BOFF[N] == OUT_COLS

C_KQ = N + 1
C_M2 = 1 + C2[N]


def _chunk_plan(total_nb, nb):
    if total_nb == 64 and nb == 16:
        return [16, 14, 12, 10, 8, 4]
    half = max(1, nb // 2)
    if total_nb > 2 * half:
        mid = total_nb - 2 * half
        plan = [half] + [nb] * (mid // nb)
        if mid % nb:
            plan.append(mid % nb)
        plan.append(half)
        return plan
    m = min(nb, total_nb)
    return [m] * (total_nb // m)


def build_program(rows=ROWS_PER_CORE, nb=NB_DEFAULT, reps=1):
    from contextlib import ExitStack
    from concourse import bacc, mybir, tile

    f32 = mybir.dt.float32
    u16 = mybir.dt.uint16
    mn = mybir.AluOpType.min
    mult = mybir.AluOpType.mult
    add = mybir.AluOpType.add
    Copy = mybir.ActivationFunctionType.Copy

    total_nb = rows // P
    assert total_nb * P == rows
    plan = _chunk_plan(total_nb, nb)
    assert sum(plan) == total_nb
    nb_max = max(plan)
    row_off = [0]
    for nbi in plan:
        row_off.append(row_off[-1] + P * nbi)

    nc = bacc.Bacc()
    xl_d = nc.declare_dram_parameter("xl", [rows, N], f32, isOutput=False)
    xu_d = nc.declare_dram_parameter("xu", [rows, N], f32, isOutput=False)
    # flat output: per chunk a [P, OUT_COLS, nb] column-major slab; the host
    # transposes back.  Contiguous 2*OUT_COLS*nb-byte run per partition.
    ok_d = nc.declare_dram_parameter(
        "out_k", [rows * OUT_COLS], u16, isOutput=True)

    def dram_views(ch):
        r0, r1 = row_off[ch], row_off[ch + 1]
        nbi = plan[ch]
        return (
            xl_d[:][r0:r1].rearrange("(nb p) t -> p nb t", p=P),
            xu_d[:][r0:r1].rearrange("(nb p) t -> p nb t", p=P),
            ok_d[:][r0 * OUT_COLS:r1 * OUT_COLS].rearrange(
                "(p x) -> p x", p=P),
            nbi,
        )

    with ExitStack() as ctx:
        tc = ctx.enter_context(tile.TileContext(nc))
        cst = ctx.enter_context(tc.tile_pool(name="cst", bufs=1))
        inp = ctx.enter_context(tc.tile_pool(name="inp", bufs=2))
        scp = ctx.enter_context(tc.tile_pool(name="scp", bufs=2))
        kp = ctx.enter_context(tc.tile_pool(name="kp", bufs=2))
        m2p = ctx.enter_context(tc.tile_pool(name="m2p", bufs=2))
        okp = ctx.enter_context(tc.tile_pool(name="okp", bufs=3))

        # one-time column-index plane: row t holds float(t)
        idxt = cst.tile([P, N * nb_max], f32, tag="idx")
        idx3 = idxt[:].rearrange("p (t nb) -> p t nb", t=N)
        for t in range(N):
            nc.gpsimd.memset(idx3[:, t:t + 1, :], float(t))

        iters = [(rep, ch) for rep in range(reps) for ch in range(len(plan))]
        in_tiles = {}

        def issue_in(i):
            _, ch_i = iters[i]
            xl_v, xu_v, _, nb_i = dram_views(ch_i)
            inb = inp.tile([P, nb_i * 2 * N], f32, tag="inb")
            in3 = inb[:].rearrange("p (v nb t) -> p v nb t", v=2, t=N)
            nc.sync.dma_start(out=in3[:, 0], in_=xl_v)
            nc.sync.dma_start(out=in3[:, 1], in_=xu_v)
            in_tiles[i] = in3

        issue_in(0)
        for it, (_rep, ch) in enumerate(iters):
            if it + 1 < len(iters):
                issue_in(it + 1)
            _, _, ok_v, nb = dram_views(ch)
            in3 = in_tiles.pop(it)

            # s~ = l + u (row-major), then transpose to column-major and
            # quantize: S = rint(2047*s~) via the fp32 magic constant
            scr = scp.tile([P, nb * 2 * N], f32, tag="scr")
            sc3 = scr[:].rearrange("p (v t nb) -> p v t nb", v=2, t=N)
            sR, sT = sc3[:, 0], sc3[:, 1]
            # writing the s-add through a transposed view lands the data
            # column-major directly -- no separate transpose pass
            nc.vector.tensor_tensor(
                sR.rearrange("p t nb -> p nb t"), in3[:, 0], in3[:, 1], add)
            nc.scalar.activation(sT, sR, Copy, bias=MAGIC, scale=S_SCALE)
            nc.scalar.activation(sR, sT, Copy, bias=-MAGIC)

            # keys: K = S*16 + t -> kt rows 1..16, sentinel row 0
            kt = kp.tile([P, C_KQ * nb], u16, tag="kt")
            k3 = kt[:].rearrange("p (q nb) -> p q nb", q=C_KQ)
            nc.gpsimd.memset(k3[:, 0:1, :], SENTINEL_U16)
            nc.vector.scalar_tensor_tensor(
                k3[:, 1:1 + N, :], sR, 16.0, idx3[:, :, :nb], mult, add)

            # pairs: M2 block j = min(prefix, broadcast single j)
            m2 = m2p.tile([P, C_M2 * nb], u16, tag="m2")
            m3 = m2[:].rearrange("p (q nb) -> p q nb", q=C_M2)
            nc.gpsimd.memset(m3[:, 0:1, :], SENTINEL_U16)
            for j in range(N):
                W = j + 1
                q0 = 1 + C2[j]
                ls = k3[:, 0:W, :]
                rs = k3[:, 1 + j:2 + j, :].to_broadcast((P, W, nb))
                nc.vector.tensor_tensor(m3[:, q0:q0 + W, :], ls, rs, mn)

            # finals: block t = min(M2 prefix, broadcast single t).
            # Store waves (~44 cols, alternating sync/gpsimd queues: ~128
            # moderate descriptors each, the pattern HW DMA engines fan
            # out) fire AS SOON AS their columns are computed, overlapping
            # stores with the remaining final mins.
            ok = okp.tile([P, OUT_COLS * nb], u16, tag="ok")
            o3 = ok[:].rearrange("p (o nb) -> p o nb", o=OUT_COLS)
            o2 = ok[:]
            step = 44
            waves = [(c0, min(c0 + step, OUT_COLS))
                     for c0 in range(0, OUT_COLS, step)]
            wi = 0

            def fire_waves(done_cols):
                nonlocal wi
                while wi < len(waves) and waves[wi][1] <= done_cols:
                    c0, c1 = waves[wi]
                    src_ap = o2[:, c0 * nb:c1 * nb]
                    dst_ap = ok_v[:, c0 * nb:c1 * nb]
                    if wi % 2 == 0:
                        nc.sync.dma_start(out=dst_ap, in_=src_ap)
                    else:
                        nc.gpsimd.dma_start(out=dst_ap, in_=src_ap)
                    wi += 1

            for t in range(N):
                W = C2[t] + 1
                b0 = BOFF[t]
                ls = m3[:, 0:W, :]
                rs = k3[:, 1 + t:2 + t, :].to_broadcast((P, W, nb))
                nc.vector.tensor_tensor(o3[:, b0:b0 + W, :], ls, rs, mn)
                fire_waves(b0 + W)
            fire_waves(OUT_COLS)

    nc.finalize()
    return nc


# ----------------------------------------------------------------------------
# Exact reference semantics in numpy (for quantization-ambiguous rows)
# ----------------------------------------------------------------------------
def _build_plan():
    from itertools import combinations

    items = list(range(N))
    index_dict = {(i,): i for i in items}
    count = N
    plan = []
    for length in range(2, min(ADD, N) + 1):
        combos = list(combinations(items, length))
        left = np.array([index_dict[c[1:]] for c in combos], dtype=np.int32)
        right = np.array([index_dict[c[:-1]] for c in combos], dtype=np.int32)
        for c in combos:
            index_dict[c] = count
            count += 1
        plan.append((left, right))

    def bitmask(c):
        m = 0
        for i in c:
            m |= 1 << i
        return m

    order = np.array(
        [index_dict[c] for c in sorted(index_dict, key=bitmask)], dtype=np.int32
    )
    return plan, order


_PLAN_CACHE = None


def _reference_numpy(xl, xu):
    global _PLAN_CACHE
    if _PLAN_CACHE is None:
        _PLAN_CACHE = _build_plan()
    plan, order = _PLAN_CACHE
    a0 = np.float32(1.0 - ALPHA)
    a1 = np.float32(ALPHA)
    b0 = np.float32(1.0 - BETA)
    b1 = np.float32(BETA)
    mat_l, mat_u = xl.astype(np.float32), xu.astype(np.float32)
    for left_idx, right_idx in plan:
        ll, lu = mat_l[:, left_idx], mat_u[:, left_idx]
        rl, ru = mat_l[:, right_idx], mat_u[:, right_idx]
        cur = a0 * ll + a1 * lu
        nxt = a0 * rl + a1 * ru
        bcur = b0 * ll + b1 * lu
        bnxt = b0 * rl + b1 * ru
        choose_right = np.where(cur == nxt, bcur > bnxt, cur > nxt)
        res_l = np.where(choose_right, rl, ll)
        res_u = np.where(choose_right, ru, lu)
        mat_l = np.concatenate([mat_l, res_l], axis=1)
        mat_u = np.concatenate([mat_u, res_u], axis=1)
    return mat_l[:, order], mat_u[:, order]


_PROGRAM_CACHE = {}


def _get_program(rows, nb):
    key = (rows, nb)
    if key not in _PROGRAM_CACHE:
        _PROGRAM_CACHE[key] = build_program(rows, nb)
    return _PROGRAM_CACHE[key]


def _decode_core(flat, rows, nb=NB_DEFAULT):
    """Per-core flat u16 slab -> row-major K [rows, OUT_COLS]."""
    plan = _chunk_plan(rows // P, nb)
    out = np.empty((rows, OUT_COLS), dtype=np.uint16)
    r0 = 0
    base = 0
    for nbi in plan:
        n = P * nbi * OUT_COLS
        slab = flat[base:base + n].reshape(P, OUT_COLS, nbi)
        # rows within the chunk are (nb p)-ordered
        out[r0:r0 + P * nbi] = slab.transpose(2, 0, 1).reshape(P * nbi, OUT_COLS)
        base += n
        r0 += P * nbi
    return out


def kernel(xl, xu):
    from concourse.bass_utils import run_bass_kernel_spmd

    xl = np.ascontiguousarray(np.asarray(xl), dtype=np.float32)
    xu = np.ascontiguousarray(np.asarray(xu), dtype=np.float32)
    assert xl.shape == (BATCH, N) and xu.shape == (BATCH, N)

    nc = _get_program(ROWS_PER_CORE, NB_DEFAULT)

    in_maps = []
    for c in range(N_CORES):
        sl = slice(c * ROWS_PER_CORE, (c + 1) * ROWS_PER_CORE)
        in_maps.append({"xl": xl[sl], "xu": xu[sl]})

    res = run_bass_kernel_spmd(nc, in_maps, list(range(N_CORES))).results

    K = np.concatenate(
        [_decode_core(r["out_k"], ROWS_PER_CORE) for r in res], axis=0)
    Ki = K.astype(np.int32)
    S = (Ki >> 4).astype(np.float32)
    idx = (Ki & 15).astype(np.int64)

    # winner values gathered EXACTLY from the original inputs
    out_l = np.take_along_axis(xl, idx, axis=1)
    out_u = np.take_along_axis(xu, idx, axis=1)

    # patch rows where any two quantized scores are within 1 (the only rows
    # where the quantized argmin can disagree with the reference compare)
    s_single = S[:, np.array(BOFF[:N], dtype=np.int64)]
    ss = np.sort(s_single, axis=1)
    bad = (np.diff(ss, axis=1) <= 1.0).any(axis=1)
    rows = np.nonzero(bad)[0]
    if rows.size:
        pl, pu = _reference_numpy(xl[rows], xu[rows])
        out_l[rows] = pl
        out_u[rows] = pu

    return out_l, out_u
